# revision 1
# baseline (speedup 1.0000x reference)
"""Causal self-attention on 8 trn2 NeuronCores.

Sharding: core c handles batch b = c // 4 and head group g = c % 4
(heads 4g..4g+3 of 16).  Each core computes:
  stage A: qkT = (W_qk_slice)^T @ x^T   (feature-major, d-major q/k)
           v   = x @ W_v_slice          (token-major, + ones column)
  stage B: per head, causal attention in S^T layout (keys on partitions,
           q on free dim): S^T = k @ q^T, P = exp(S/8) * mask,
           pv = [v | 1]^T @ P^T  -> rows 0..63 = out^T, row 64 = denom
           z = out^T / denom  (feature-major attention output)
  stage C: y_partial = z^T @ W_proj[row slice]   (token-major)
Host sums the 4 partials per batch and adds b_proj.

Matmul operands are bf16 (fp32 PSUM accumulation); the softmax
denominator reciprocal is kept at fp32/fp32r precision.
"""

import numpy as np

B, T, C, H, D = 2, 2048, 1024, 16, 64
HPC = 4              # heads per core
FW = HPC * D         # 256 attention-output features per core
QKF = 2 * FW         # 512 q+k features per core
NTW = T // 512       # 4 q/token windows of 512
NTT = T // 128       # 16 token tiles of 128
NKC = C // 128       # 8 contraction chunks for stage A

_CACHE = {}


def _build_nc(debug_outputs=False):
    import concourse.bass as bass  # noqa: F401
    import concourse.mybir as mybir
    import concourse.tile as tile
    from concourse import bacc
    from contextlib import ExitStack

    f32 = mybir.dt.float32
    r32 = mybir.dt.float32r
    bf16 = mybir.dt.bfloat16
    AF = mybir.ActivationFunctionType

    nc = bacc.Bacc(None, target_bir_lowering=False)
    xT = nc.declare_dram_parameter("xT", [C, T], bf16, isOutput=False)
    w_qk = nc.declare_dram_parameter("w_qk", [C, QKF], bf16, isOutput=False)
    b_qk = nc.declare_dram_parameter("b_qk", [QKF], f32, isOutput=False)
    w_v = nc.declare_dram_parameter("w_v", [C, FW], bf16, isOutput=False)
    b_v = nc.declare_dram_parameter("b_v", [FW], bf16, isOutput=False)
    w_p = nc.declare_dram_parameter("w_p", [FW, C], bf16, isOutput=False)
    masks = nc.declare_dram_parameter("masks", [4, 128, 512], bf16, isOutput=False)
    y = nc.declare_dram_parameter("y", [T, C], f32, isOutput=True)
    if debug_outputs:
        qkT_dbg = nc.declare_dram_parameter("qkT_dbg", [128, 4, T], bf16, isOutput=True)
        v_dbg = nc.declare_dram_parameter("v_dbg", [128, NTT, HPC * 2 * D], bf16, isOutput=True)
        z_dbg = nc.declare_dram_parameter("z_dbg", [128, 2, T], bf16, isOutput=True)

    with nc.allow_low_precision(reason="bf16 matmul dataflow"), \
            tile.TileContext(nc) as tc, ExitStack() as ctx:
        wpool = ctx.enter_context(tc.tile_pool(name="wpool", bufs=1))
        big = ctx.enter_context(tc.tile_pool(name="big", bufs=1))
        xw = ctx.enter_context(tc.tile_pool(name="xw", bufs=16))
        ptp = ctx.enter_context(tc.tile_pool(name="ptp", bufs=8))
        smalls = ctx.enter_context(tc.tile_pool(name="smalls", bufs=4))
        ydr = ctx.enter_context(tc.tile_pool(name="ydr", bufs=4))
        upool = ctx.enter_context(tc.tile_pool(name="upool", bufs=8))
        ps = ctx.enter_context(tc.tile_pool(name="ps", bufs=3, space="PSUM"))
        psv = ctx.enter_context(tc.tile_pool(name="psv", bufs=1, space="PSUM"))
        ppv = ctx.enter_context(tc.tile_pool(name="ppv", bufs=4, space="PSUM"))

        # ---- constants / weights to SBUF ----
        w_qk_sb = wpool.tile([128, NKC, QKF], bf16)
        nc.sync.dma_start(out=w_qk_sb, in_=w_qk.rearrange("(kc p) f -> p kc f", p=128))
        w_v_sb = wpool.tile([128, NKC, FW], bf16)
        nc.sync.dma_start(out=w_v_sb, in_=w_v.rearrange("(kc p) f -> p kc f", p=128))
        w_p_sb = wpool.tile([128, 2, C], bf16)
        nc.sync.dma_start(out=w_p_sb, in_=w_p.rearrange("(fc p) o -> p fc o", p=128))
        b_qk_sb = wpool.tile([128, 4], f32)
        nc.sync.dma_start(out=b_qk_sb, in_=b_qk.rearrange("(f p) -> p f", p=128))
        b_v_sb = wpool.tile([1, FW], bf16)
        nc.sync.dma_start(out=b_v_sb, in_=b_v[None, :])
        masks_sb = wpool.tile([128, 4, 512], bf16)
        nc.sync.dma_start(out=masks_sb, in_=masks.rearrange("j p q -> p j q"))
        ones_f = wpool.tile([1, 128], f32)
        nc.vector.memset(ones_f, 1.0)
        ones_bf = wpool.tile([1, 128], bf16)
        nc.vector.tensor_copy(ones_bf, ones_f)
        onecol_f = wpool.tile([128, 1], f32)
        nc.vector.memset(onecol_f, 1.0)

        qkT_sb = big.tile([128, 4, T], bf16)         # chunks: q01 q23 k01 k23
        v_sb = big.tile([128, NTT, HPC * 2 * D], bf16)
        all_ones_cols = v_sb.rearrange("p t (h e) -> p t h e", e=2 * D)[:, :, :, D:]
        nc.vector.tensor_copy(
            all_ones_cols,
            onecol_f[:, :, None, None].broadcast_to([128, NTT, HPC, D]),
        )
        z_sb = big.tile([128, 2, T], bf16)

        # ---- interleaved per token-window: A(tw), B(tw), C(tw) ----
        def stage_a(tw):
            xts = []
            for kc in range(NKC):
                xt = xw.tile([128, 512], bf16, tag="xt", name=f"xt_{tw}_{kc}")
                nc.sync.dma_start(
                    out=xt, in_=xT[kc * 128:(kc + 1) * 128, tw * 512:(tw + 1) * 512]
                )
                xts.append(xt)
            for f in range(4):
                ps_qk = ps.tile([128, 512], f32, tag="ps", name=f"psqk_{tw}_{f}")
                for kc in range(NKC):
                    nc.tensor.matmul(
                        ps_qk,
                        lhsT=w_qk_sb[:, kc, f * 128:(f + 1) * 128],
                        rhs=xts[kc],
                        start=(kc == 0),
                        stop=(kc == NKC - 1),
                    )
                nc.scalar.activation(
                    out=qkT_sb[:, f, tw * 512:(tw + 1) * 512],
                    in_=ps_qk,
                    func=AF.Identity,
                    bias=b_qk_sb[:, f:f + 1],
                    scale=1.0,
                )
            for t4 in range(4):
                tt = tw * 4 + t4
                ps_v = psv.tile([128, FW], f32, tag="psv", name=f"psv_{tt}")
                for kc in range(NKC):
                    nc.tensor.matmul(
                        ps_v,
                        lhsT=xts[kc][:, t4 * 128:(t4 + 1) * 128],
                        rhs=w_v_sb[:, kc, :],
                        start=(kc == 0),
                        stop=False,
                    )
                nc.tensor.matmul(
                    ps_v,
                    lhsT=ones_bf,
                    rhs=b_v_sb,
                    start=False,
                    stop=True,
                )
                for h in range(HPC):
                    nc.scalar.copy(
                        v_sb[:, tt, h * 2 * D:h * 2 * D + D],
                        ps_v[:, h * D:(h + 1) * D],
                    )

        def stage_b(hp, tw):
                nkc = 4 * (tw + 1)
                pvs = []
                for hh in range(2):
                    pv_t = ppv.tile([2 * D, 512], f32, tag="pv", name=f"pv_{hp}_{tw}_{hh}")
                    pvs.append(pv_t)
                for kc in range(nkc):
                    j = kc - 4 * tw
                    sts, pts = [], []
                    for hh in range(2):
                        h = 2 * hp + hh
                        base = 64 * (h % 2)
                        fq = h // 2
                        fk = 2 + h // 2
                        st = ps.tile([128, 512], f32, tag="ps", name=f"st_{hp}_{tw}_{kc}_{hh}")
                        nc.tensor.matmul(
                            st,
                            lhsT=qkT_sb[base:base + 64, fk, kc * 128:(kc + 1) * 128],
                            rhs=qkT_sb[base:base + 64, fq, tw * 512:(tw + 1) * 512],
                            start=True,
                            stop=True,
                        )
                        sts.append(st)
                    for hh in range(2):
                        pt = ptp.tile([128, 512], bf16, tag="pt", name=f"pt_{hp}_{tw}_{kc}_{hh}")
                        nc.scalar.activation(out=pt, in_=sts[hh], func=AF.Exp, scale=0.125)
                        if j >= 0:
                            nc.vector.tensor_mul(pt, pt, masks_sb[:, j, :])
                        pts.append(pt)
                    for hh in range(2):
                        h = 2 * hp + hh
                        nc.tensor.matmul(
                            pvs[hh],
                            lhsT=v_sb[:, kc, h * 2 * D:(h + 1) * 2 * D],
                            rhs=pts[hh],
                            start=(kc == 0),
                            stop=(kc == nkc - 1),
                        )
                return pvs

        def stage_b_epi(hp, tw, pvs):
                us = []
                for hh in range(2):
                    u = upool.tile([D + 1, 512], f32, tag="u", name=f"u_{hp}_{tw}_{hh}")
                    nc.vector.tensor_copy(u, pvs[hh][0:D + 1, :])
                    us.append(u)
                for hh in range(2):
                    h = 2 * hp + hh
                    rec_f = smalls.tile([1, 512], f32, tag="recf", name=f"recf_{hp}_{tw}_{hh}")
                    nc.vector.reciprocal(rec_f, us[hh][D:D + 1, :])
                    Rb = smalls.tile([64, 512], f32, tag="Rb", name=f"Rb_{hp}_{tw}_{hh}")
                    nc.gpsimd.partition_broadcast(Rb, rec_f)
                    nc.vector.tensor_mul(
                        z_sb[64 * (h % 2):64 * (h % 2) + 64, h // 2, tw * 512:(tw + 1) * 512],
                        us[hh][0:D, :],
                        Rb,
                    )

        def stage_c(tt):
            for n in range(2):
                ps_y = ps.tile([128, 512], f32, tag="ps", name=f"psy_{tt}_{n}")
                for fc in range(2):
                    nc.tensor.matmul(
                        ps_y,
                        lhsT=z_sb[:, fc, tt * 128:(tt + 1) * 128],
                        rhs=w_p_sb[:, fc, n * 512:(n + 1) * 512],
                        start=(fc == 0),
                        stop=(fc == 1),
                    )
                yt = ydr.tile([128, 512], f32, tag="yt", name=f"yt_{tt}_{n}")
                nc.scalar.copy(yt, ps_y)
                nc.sync.dma_start(
                    out=y[tt * 128:(tt + 1) * 128, n * 512:(n + 1) * 512], in_=yt
                )

        for tw in range(NTW):
            stage_a(tw)
            pvs0 = stage_b(0, tw)
            pvs1 = stage_b(1, tw)
            if tw > 0:
                for t4 in range(4):
                    stage_c((tw - 1) * 4 + t4)
            stage_b_epi(0, tw, pvs0)
            stage_b_epi(1, tw, pvs1)
        for t4 in range(4):
            stage_c(3 * 4 + t4)

        if debug_outputs:
            nc.sync.dma_start(out=qkT_dbg[:, :, :], in_=qkT_sb)
            nc.sync.dma_start(out=v_dbg[:, :, :], in_=v_sb)
            nc.sync.dma_start(out=z_dbg[:, :, :], in_=z_sb)
    nc.finalize()
    return nc


def _causal_masks():
    j = np.arange(4)[:, None, None]
    p = np.arange(128)[None, :, None]
    q = np.arange(512)[None, None, :]
    return (q >= 128 * j + p).astype(np.float32)


def make_in_maps(x, W_attn, b_attn, W_proj):
    import ml_dtypes

    bf = ml_dtypes.bfloat16
    masks = _causal_masks().astype(bf)
    xT = [np.ascontiguousarray(x[b].T).astype(bf) for b in range(B)]
    in_maps = []
    for c in range(8):
        b, g = c // 4, c % 4
        heads = [4 * g + i for i in range(HPC)]
        wq = np.concatenate([W_attn[:, 64 * h:64 * h + 64] for h in heads], axis=1)
        wk = np.concatenate([W_attn[:, C + 64 * h:C + 64 * h + 64] for h in heads], axis=1)
        wv = np.concatenate([W_attn[:, 2 * C + 64 * h:2 * C + 64 * h + 64] for h in heads], axis=1)
        bq = np.concatenate([b_attn[64 * h:64 * h + 64] for h in heads])
        bk = np.concatenate([b_attn[C + 64 * h:C + 64 * h + 64] for h in heads])
        bv = np.concatenate([b_attn[2 * C + 64 * h:2 * C + 64 * h + 64] for h in heads])
        in_maps.append({
            "xT": xT[b],
            "w_qk": np.ascontiguousarray(np.concatenate([wq, wk], axis=1)).astype(bf),
            "b_qk": np.ascontiguousarray(np.concatenate([bq, bk]), dtype=np.float32),
            "w_v": np.ascontiguousarray(wv).astype(bf),
            "b_v": np.ascontiguousarray(bv).astype(bf),
            "w_p": np.ascontiguousarray(W_proj[FW * g:FW * (g + 1), :]).astype(bf),
            "masks": masks,
        })
    return in_maps


def assemble(results, b_proj):
    y = np.zeros((B, T, C), np.float32)
    for c in range(8):
        y[c // 4] += results[c]["y"]
    y += b_proj[None, None, :].astype(np.float32)
    return y


def kernel(**inputs):
    from concourse.bass_utils import run_bass_kernel_spmd

    x = np.asarray(inputs["x"], np.float32)
    W_attn = np.asarray(inputs["W_attn"], np.float32)
    b_attn = np.asarray(inputs["b_attn"], np.float32)
    W_proj = np.asarray(inputs["W_proj"], np.float32)
    b_proj = np.asarray(inputs["b_proj"], np.float32)

    if "nc" not in _CACHE:
        _CACHE["nc"] = _build_nc()
    nc = _CACHE["nc"]
    in_maps = make_in_maps(x, W_attn, b_attn, W_proj)
    res = run_bass_kernel_spmd(nc, in_maps, core_ids=list(range(8))).results
    return assemble(res, b_proj)



# revision 4
# speedup vs baseline: 1.0176x; 1.0176x over previous
"""Causal self-attention on 8 trn2 NeuronCores.

Sharding: core c handles batch b = c // 4 and head group g = c % 4
(heads 4g..4g+3 of 16).  Each core computes:
  stage A: qkT = (W_qk_slice)^T @ x^T   (feature-major, d-major q/k)
           v   = x @ W_v_slice          (token-major, + ones column)
  stage B: per head, causal attention in S^T layout (keys on partitions,
           q on free dim): S^T = k @ q^T, P = exp(S/8) * tri-mask,
           pv = [v | 1]^T @ P^T  -> rows 0..63 = out^T, row 64 = denom
           z = out^T * recip(denom)  (feature-major attention output)
  stage C: y_partial = z^T @ W_proj[row slice]   (token-major, bf16 out)
Host sums the 4 partials per batch and adds b_proj.

v2 layout: score tiles for the two heads of an hp-group share one
2-bank PSUM "pair" tile (one exp per pair), diagonal chunks are
causally trimmed to q >= 128*j, the softmax denominator uses the fast
DVE approx reciprocal straight out of PSUM, weight loads are chunked so
the first matmul starts early, and stage C streams bf16.
"""

import numpy as np

B, T, C, H, D = 2, 2048, 1024, 16, 64
HPC = 4              # heads per core
FW = HPC * D         # 256 attention-output features per core
QKF = 2 * FW         # 512 q+k features per core
NTW = T // 512       # 4 q/token windows of 512
NTT = T // 128       # 16 token tiles of 128
NKC = C // 128       # 8 contraction chunks for stage A

_CACHE = {}


def _build_nc(debug_outputs=False):
    import concourse.bass as bass  # noqa: F401
    import concourse.mybir as mybir
    import concourse.tile as tile
    from concourse import bacc
    from contextlib import ExitStack

    f32 = mybir.dt.float32
    bf16 = mybir.dt.bfloat16
    AF = mybir.ActivationFunctionType

    nc = bacc.Bacc(None, target_bir_lowering=False)
    xT = nc.declare_dram_parameter("xT", [C, T], bf16, isOutput=False)
    w_qk = nc.declare_dram_parameter("w_qk", [C, QKF], bf16, isOutput=False)
    b_qk = nc.declare_dram_parameter("b_qk", [QKF], f32, isOutput=False)
    w_v = nc.declare_dram_parameter("w_v", [C, FW], bf16, isOutput=False)
    b_v = nc.declare_dram_parameter("b_v", [FW], bf16, isOutput=False)
    w_p = nc.declare_dram_parameter("w_p", [FW, C], bf16, isOutput=False)
    tri = nc.declare_dram_parameter("tri", [128, 128], bf16, isOutput=False)
    y = nc.declare_dram_parameter("y", [T, C], bf16, isOutput=True)
    if debug_outputs:
        qkT_dbg = nc.declare_dram_parameter("qkT_dbg", [128, 4, T], bf16, isOutput=True)
        v_dbg = nc.declare_dram_parameter("v_dbg", [128, NTT, HPC * (D + 1)], bf16, isOutput=True)
        z_dbg = nc.declare_dram_parameter("z_dbg", [128, 2, T], bf16, isOutput=True)

    with nc.allow_low_precision(reason="bf16 matmul dataflow"), \
            tile.TileContext(nc) as tc, ExitStack() as ctx:
        wpool = ctx.enter_context(tc.tile_pool(name="wpool", bufs=1))
        big = ctx.enter_context(tc.tile_pool(name="big", bufs=1))
        xw = ctx.enter_context(tc.tile_pool(name="xw", bufs=2))
        ptp = ctx.enter_context(tc.tile_pool(name="ptp", bufs=4))
        smalls = ctx.enter_context(tc.tile_pool(name="smalls", bufs=4))
        ytp = ctx.enter_context(tc.tile_pool(name="ytp", bufs=2))
        pairs = ctx.enter_context(tc.tile_pool(name="pairs", bufs=2, space="PSUM"))
        ppv = ctx.enter_context(tc.tile_pool(name="ppv", bufs=4, space="PSUM"))

        # ---- weights / constants ----
        w_qk_sb = wpool.tile([128, NKC, QKF], bf16)
        w_v_sb = wpool.tile([128, NKC, FW], bf16)
        w_p_sb = wpool.tile([128, 2, C], bf16)
        b_qk_sb = wpool.tile([128, 4], f32)
        b_v_sb = wpool.tile([1, FW], bf16)
        tri_sb = wpool.tile([128, 128], bf16)
        ones_f = wpool.tile([1, 128], f32)
        ones_bf = wpool.tile([1, 128], bf16)
        onecol_f = wpool.tile([128, 1], f32)

        xts = {}

        def prefetch_x(tw):
            xt = xw.tile([128, NKC, 512], bf16, tag="xt", name=f"xt_{tw}")
            for kc in range(NKC):
                nc.sync.dma_start(
                    out=xt[:, kc, :],
                    in_=xT[kc * 128:(kc + 1) * 128, tw * 512:(tw + 1) * 512],
                )
            xts[tw] = xt

        # interleave the first window's x chunks with the qk weight chunks
        # so the kc-ascending accumulation can start as soon as chunk 0 lands
        xt0 = xw.tile([128, NKC, 512], bf16, tag="xt", name="xt_0")
        for kc in range(NKC):
            nc.sync.dma_start(out=w_qk_sb[:, kc, :], in_=w_qk[kc * 128:(kc + 1) * 128, :])
            nc.sync.dma_start(out=xt0[:, kc, :], in_=xT[kc * 128:(kc + 1) * 128, 0:512])
        xts[0] = xt0
        nc.sync.dma_start(out=b_qk_sb, in_=b_qk.rearrange("(f p) -> p f", p=128))
        for kc in range(NKC):
            nc.sync.dma_start(out=w_v_sb[:, kc, :], in_=w_v[kc * 128:(kc + 1) * 128, :])
        nc.sync.dma_start(out=b_v_sb, in_=b_v[None, :])
        nc.sync.dma_start(out=tri_sb, in_=tri[:, :])
        nc.sync.dma_start(out=w_p_sb, in_=w_p.rearrange("(fc p) o -> p fc o", p=128))

        nc.vector.memset(ones_f, 1.0)
        nc.vector.tensor_copy(ones_bf, ones_f)
        nc.vector.memset(onecol_f, 1.0)

        qkT_sb = big.tile([128, 4, T], bf16)         # chunks: q01 q23 k01 k23
        v_sb = big.tile([128, NTT, HPC, D + 1], bf16)
        nc.vector.tensor_copy(
            v_sb[:, :, :, D:],
            onecol_f[:, :, None, None].broadcast_to([128, NTT, HPC, 1]),
        )
        z_sb = big.tile([128, 2, T], bf16)

        def stage_a(tw):
            xt = xts[tw]
            for fp in range(2):
                pqk = pairs.tile([128, 2, 512], f32, tag="pair", name=f"pqk_{tw}_{fp}")
                for half in range(2):
                    f = 2 * fp + half
                    for kc in range(NKC):
                        nc.tensor.matmul(
                            pqk[:, half, :],
                            lhsT=w_qk_sb[:, kc, f * 128:(f + 1) * 128],
                            rhs=xt[:, kc, :],
                            start=(kc == 0),
                            stop=(kc == NKC - 1),
                        )
                for half in range(2):
                    f = 2 * fp + half
                    nc.scalar.activation(
                        out=qkT_sb[:, f, tw * 512:(tw + 1) * 512],
                        in_=pqk[:, half, :],
                        func=AF.Identity,
                        bias=b_qk_sb[:, f:f + 1],
                        scale=1.0,
                    )
            pv4 = pairs.tile([128, 4, FW], f32, tag="pair", name=f"pv4_{tw}")
            for t4 in range(4):
                for kc in range(NKC):
                    nc.tensor.matmul(
                        pv4[:, t4, :],
                        lhsT=xt[:, kc, t4 * 128:(t4 + 1) * 128],
                        rhs=w_v_sb[:, kc, :],
                        start=(kc == 0),
                        stop=False,
                    )
                nc.tensor.matmul(
                    pv4[:, t4, :], lhsT=ones_bf, rhs=b_v_sb, start=False, stop=True
                )
            nc.scalar.copy(
                v_sb[:, tw * 4:(tw + 1) * 4, :, 0:D],
                pv4.rearrange("p t (h d) -> p t h d", d=D),
            )

        def stage_b(hp, tw):
            nkc = 4 * (tw + 1)
            pvs = []
            for hh in range(2):
                pv_t = ppv.tile([D + 1, 512], f32, tag="pv", name=f"pv_{hp}_{tw}_{hh}")
                pvs.append(pv_t)
            for kc in range(nkc):
                j = kc - 4 * tw
                q0 = 128 * j if j > 0 else 0
                stp = pairs.tile([128, 2, 512], f32, tag="pair", name=f"st_{hp}_{tw}_{kc}")
                for hh in range(2):
                    h = 2 * hp + hh
                    base = 64 * (h % 2)
                    fq = h // 2
                    fk = 2 + h // 2
                    nc.tensor.matmul(
                        stp[:, hh, q0:],
                        lhsT=qkT_sb[base:base + 64, fk, kc * 128:(kc + 1) * 128],
                        rhs=qkT_sb[base:base + 64, fq, tw * 512 + q0:(tw + 1) * 512],
                        start=True,
                        stop=True,
                    )
                pt = ptp.tile([128, 2, 512], bf16, tag="pt", name=f"pt_{hp}_{tw}_{kc}")
                nc.scalar.activation(
                    out=pt[:, :, q0:], in_=stp[:, :, q0:], func=AF.Exp, scale=0.125
                )
                if j >= 0:
                    nc.vector.tensor_mul(
                        pt[:, :, 128 * j:128 * (j + 1)],
                        pt[:, :, 128 * j:128 * (j + 1)],
                        tri_sb[:, None, :].broadcast_to([128, 2, 128]),
                    )
                for hh in range(2):
                    h = 2 * hp + hh
                    nc.tensor.matmul(
                        pvs[hh][:, q0:],
                        lhsT=v_sb[:, kc, h, :],
                        rhs=pt[:, hh, q0:],
                        start=(kc == 0),
                        stop=(kc == nkc - 1),
                    )
            return pvs

        def stage_b_epi(hp, tw, pvs):
            for hh in range(2):
                h = 2 * hp + hh
                u = smalls.tile([D + 1, 512], f32, tag="u", name=f"u_{hp}_{tw}_{hh}", bufs=2)
                nc.vector.tensor_copy(u, pvs[hh][0:D + 1, :])
                rec = smalls.tile([1, 512], f32, tag="rec", name=f"rec_{hp}_{tw}_{hh}")
                nc.vector.reciprocal(rec, u[D:D + 1, :])
                Rb = smalls.tile([64, 512], f32, tag="Rb", name=f"Rb_{hp}_{tw}_{hh}", bufs=2)
                nc.gpsimd.partition_broadcast(Rb, rec)
                nc.vector.tensor_mul(
                    z_sb[64 * (h % 2):64 * (h % 2) + 64, h // 2, tw * 512:(tw + 1) * 512],
                    u[0:D, :],
                    Rb,
                )

        def stage_c(tt):
            psy = pairs.tile([128, 2, 512], f32, tag="pair", name=f"psy_{tt}")
            for n in range(2):
                for fc in range(2):
                    nc.tensor.matmul(
                        psy[:, n, :],
                        lhsT=z_sb[:, fc, tt * 128:(tt + 1) * 128],
                        rhs=w_p_sb[:, fc, n * 512:(n + 1) * 512],
                        start=(fc == 0),
                        stop=(fc == 1),
                    )
            yt = ytp.tile([128, 2, 512], bf16, tag="yt", name=f"yt_{tt}")
            nc.vector.tensor_copy(yt, psy)
            nc.sync.dma_start(
                out=y[tt * 128:(tt + 1) * 128, :], in_=yt.rearrange("p a b -> p (a b)")
            )

        for tw in range(NTW):
            if tw + 1 < NTW:
                prefetch_x(tw + 1)
            stage_a(tw)
            if tw > 0:
                for t4 in range(4):
                    stage_c((tw - 1) * 4 + t4)
            pvs0 = stage_b(0, tw)
            pvs1 = stage_b(1, tw)
            stage_b_epi(0, tw, pvs0)
            stage_b_epi(1, tw, pvs1)
        for t4 in range(4):
            stage_c(3 * 4 + t4)

        if debug_outputs:
            nc.sync.dma_start(out=qkT_dbg[:, :, :], in_=qkT_sb)
            nc.sync.dma_start(
                out=v_dbg[:, :, :], in_=v_sb.rearrange("p t h e -> p t (h e)")
            )
            nc.sync.dma_start(out=z_dbg[:, :, :], in_=z_sb)
    nc.finalize()
    return nc


def make_in_maps(x, W_attn, b_attn, W_proj):
    import ml_dtypes

    bf = ml_dtypes.bfloat16
    p = np.arange(128)[:, None]
    q = np.arange(128)[None, :]
    tri = (q >= p).astype(np.float32).astype(bf)
    xT = [np.ascontiguousarray(x[b].T).astype(bf) for b in range(B)]
    in_maps = []
    for c in range(8):
        b, g = c // 4, c % 4
        heads = [4 * g + i for i in range(HPC)]
        wq = np.concatenate([W_attn[:, 64 * h:64 * h + 64] for h in heads], axis=1)
        wk = np.concatenate([W_attn[:, C + 64 * h:C + 64 * h + 64] for h in heads], axis=1)
        wv = np.concatenate([W_attn[:, 2 * C + 64 * h:2 * C + 64 * h + 64] for h in heads], axis=1)
        bq = np.concatenate([b_attn[64 * h:64 * h + 64] for h in heads])
        bk = np.concatenate([b_attn[C + 64 * h:C + 64 * h + 64] for h in heads])
        bv = np.concatenate([b_attn[2 * C + 64 * h:2 * C + 64 * h + 64] for h in heads])
        in_maps.append({
            "xT": xT[b],
            "w_qk": np.ascontiguousarray(np.concatenate([wq, wk], axis=1)).astype(bf),
            "b_qk": np.ascontiguousarray(np.concatenate([bq, bk]), dtype=np.float32),
            "w_v": np.ascontiguousarray(wv).astype(bf),
            "b_v": np.ascontiguousarray(bv).astype(bf),
            "w_p": np.ascontiguousarray(W_proj[FW * g:FW * (g + 1), :]).astype(bf),
            "tri": tri,
        })
    return in_maps


def assemble(results, b_proj):
    y = np.zeros((B, T, C), np.float32)
    for c in range(8):
        y[c // 4] += np.asarray(results[c]["y"], dtype=np.float32)
    y += b_proj[None, None, :].astype(np.float32)
    return y


def kernel(**inputs):
    from concourse.bass_utils import run_bass_kernel_spmd

    x = np.asarray(inputs["x"], np.float32)
    W_attn = np.asarray(inputs["W_attn"], np.float32)
    b_attn = np.asarray(inputs["b_attn"], np.float32)
    W_proj = np.asarray(inputs["W_proj"], np.float32)
    b_proj = np.asarray(inputs["b_proj"], np.float32)

    if "nc" not in _CACHE:
        _CACHE["nc"] = _build_nc()
    nc = _CACHE["nc"]
    in_maps = make_in_maps(x, W_attn, b_attn, W_proj)
    res = run_bass_kernel_spmd(nc, in_maps, core_ids=list(range(8))).results
    return assemble(res, b_proj)


# revision 12
# speedup vs baseline: 1.1896x; 1.1690x over previous
"""Causal self-attention on 8 trn2 NeuronCores.

Sharding: core c handles batch b = c // 4 and head group g = c % 4
(heads 4g..4g+3 of 16).  Each core computes:
  stage A: qkT = (W_qk_slice)^T @ x^T   (feature-major, d-major q/k)
           v   = x @ W_v_slice          (token-major, + ones column)
  stage B: per head, causal attention in S^T layout (keys on partitions,
           q on free dim): S^T = k @ q^T, P = exp(S/8) * tri-mask,
           pv = [v | 1]^T @ P^T  -> rows 0..63 = out^T, row 64 = denom
           z = out^T * recip(denom)  (feature-major attention output)
  stage C: y_partial = z^T @ W_proj[row slice]   (token-major, bf16 out)
Host sums the 4 partials per batch and adds b_proj.

v2 layout: score tiles for the two heads of an hp-group share one
2-bank PSUM "pair" tile (one exp per pair), diagonal chunks are
causally trimmed to q >= 128*j, the softmax denominator uses the fast
DVE approx reciprocal straight out of PSUM, weight loads are chunked so
the first matmul starts early, and stage C streams bf16.
"""

import numpy as np

B, T, C, H, D = 2, 2048, 1024, 16, 64
HPC = 4              # heads per core
FW = HPC * D         # 256 attention-output features per core
QKF = 2 * FW         # 512 q+k features per core
NTW = T // 512       # 4 q/token windows of 512
NTT = T // 128       # 16 token tiles of 128
NKC = C // 128       # 8 contraction chunks for stage A

_CACHE = {}


def _build_nc(debug_outputs=False):
    import concourse.bass as bass  # noqa: F401
    import concourse.mybir as mybir
    import concourse.tile as tile
    from concourse import bacc
    from contextlib import ExitStack

    f32 = mybir.dt.float32
    bf16 = mybir.dt.bfloat16
    AF = mybir.ActivationFunctionType

    nc = bacc.Bacc(None, target_bir_lowering=False)
    xT = nc.declare_dram_parameter("xT", [C, T], bf16, isOutput=False)
    w_qk = nc.declare_dram_parameter("w_qk", [C, QKF], bf16, isOutput=False)
    b_qk = nc.declare_dram_parameter("b_qk", [QKF], f32, isOutput=False)
    w_v = nc.declare_dram_parameter("w_v", [C, FW], bf16, isOutput=False)
    b_v = nc.declare_dram_parameter("b_v", [FW], bf16, isOutput=False)
    w_p = nc.declare_dram_parameter("w_p", [FW, C], bf16, isOutput=False)
    tri = nc.declare_dram_parameter("tri", [128, 128], bf16, isOutput=False)
    y = nc.declare_dram_parameter("y", [T, C], bf16, isOutput=True)
    if debug_outputs:
        qkT_dbg = nc.declare_dram_parameter("qkT_dbg", [128, 4, T], bf16, isOutput=True)
        v_dbg = nc.declare_dram_parameter("v_dbg", [128, NTT, HPC * (D + 1)], bf16, isOutput=True)
        z_dbg = nc.declare_dram_parameter("z_dbg", [128, 2, T], bf16, isOutput=True)

    with nc.allow_low_precision(reason="bf16 matmul dataflow"), \
            tile.TileContext(nc) as tc, ExitStack() as ctx:
        wpool = ctx.enter_context(tc.tile_pool(name="wpool", bufs=1))
        big = ctx.enter_context(tc.tile_pool(name="big", bufs=1))
        xw = ctx.enter_context(tc.tile_pool(name="xw", bufs=2))
        ptp = ctx.enter_context(tc.tile_pool(name="ptp", bufs=4))
        smalls = ctx.enter_context(tc.tile_pool(name="smalls", bufs=4))
        ytp = ctx.enter_context(tc.tile_pool(name="ytp", bufs=2))
        pairs = ctx.enter_context(tc.tile_pool(name="pairs", bufs=2, space="PSUM"))
        ppv = ctx.enter_context(tc.tile_pool(name="ppv", bufs=4, space="PSUM"))

        # ---- weights / constants ----
        w_qk_sb = wpool.tile([128, NKC, QKF], bf16)
        w_v_sb = wpool.tile([128, NKC, FW], bf16)
        w_p_sb = wpool.tile([128, 2, C], bf16)
        b_qk_sb = wpool.tile([128, 4], f32)
        b_v_sb = wpool.tile([1, FW], bf16)
        tri_sb = wpool.tile([128, 128], bf16)
        ones_f = wpool.tile([1, 128], f32)
        ones_bf = wpool.tile([1, 128], bf16)
        onecol_f = wpool.tile([128, 1], f32)

        xts = {}

        def prefetch_x(tw):
            xt = xw.tile([128, NKC, 512], bf16, tag="xt", name=f"xt_{tw}")
            for c4 in range(2):
                kc = 4 * c4
                nc.sync.dma_start(
                    out=xt[:, kc:kc + 4, :],
                    in_=xT.rearrange("(kc p) t -> p kc t", p=128)[
                        :, kc:kc + 4, tw * 512:(tw + 1) * 512
                    ],
                )
            xts[tw] = xt

        # interleave the first window's x chunks with the qk weight chunks,
        # splitting the DMA *issue* load across the sync and scalar queues so
        # the kc-ascending accumulation can start after ~2 issues
        xt0 = xw.tile([128, NKC, 512], bf16, tag="xt", name="xt_0")
        for c2 in range(4):
            kc = 2 * c2
            nc.sync.dma_start(
                out=w_qk_sb[:, kc:kc + 2, :],
                in_=w_qk.rearrange("(kc p) f -> p kc f", p=128)[:, kc:kc + 2, :],
            )
            nc.scalar.dma_start(
                out=xt0[:, kc:kc + 2, :],
                in_=xT.rearrange("(kc p) t -> p kc t", p=128)[:, kc:kc + 2, 0:512],
            )
        xts[0] = xt0
        nc.scalar.dma_start(out=b_qk_sb, in_=b_qk.rearrange("(f p) -> p f", p=128))
        for c4 in range(2):
            kc = 4 * c4
            nc.sync.dma_start(
                out=w_v_sb[:, kc:kc + 4, :],
                in_=w_v.rearrange("(kc p) f -> p kc f", p=128)[:, kc:kc + 4, :],
            )
        nc.sync.dma_start(out=b_v_sb, in_=b_v[None, :])
        nc.sync.dma_start(out=tri_sb, in_=tri[:, :])
        nc.sync.dma_start(out=w_p_sb, in_=w_p.rearrange("(fc p) o -> p fc o", p=128))

        nc.vector.memset(ones_f, 1.0)
        nc.vector.tensor_copy(ones_bf, ones_f)
        nc.vector.memset(onecol_f, 1.0)

        qkT_sb = big.tile([128, 4, T], bf16)         # chunks: q01 q23 k01 k23
        # per head: [v(64) | ones] so the pv matmul puts the softmax
        # denominator on partition 64 (partition offsets must be 0/32/64/96)
        v_sb = big.tile([128, NTT, HPC, D + 1], bf16)
        nc.vector.tensor_copy(
            v_sb[:, :, :, D:],
            onecol_f[:, :, None, None].broadcast_to([128, NTT, HPC, 1]),
        )
        z_sb = big.tile([128, 2, T], bf16)

        def stage_a(tw):
            xt = xts[tw]
            for fp in range(2):
                pqk = pairs.tile([128, 2, 512], f32, tag="pair", name=f"pqk_{tw}_{fp}")
                for half in range(2):
                    f = 2 * fp + half
                    for kc in range(NKC):
                        nc.tensor.matmul(
                            pqk[:, half, :],
                            lhsT=w_qk_sb[:, kc, f * 128:(f + 1) * 128],
                            rhs=xt[:, kc, :],
                            start=(kc == 0),
                            stop=(kc == NKC - 1),
                        )
                for half in range(2):
                    f = 2 * fp + half
                    nc.scalar.activation(
                        out=qkT_sb[:, f, tw * 512:(tw + 1) * 512],
                        in_=pqk[:, half, :],
                        func=AF.Identity,
                        bias=b_qk_sb[:, f:f + 1],
                        scale=1.0,
                    )
            pv4 = pairs.tile([128, 4, FW], f32, tag="pair", name=f"pv4_{tw}")
            for t4 in range(4):
                for kc in range(NKC):
                    nc.tensor.matmul(
                        pv4[:, t4, :],
                        lhsT=xt[:, kc, t4 * 128:(t4 + 1) * 128],
                        rhs=w_v_sb[:, kc, :],
                        start=(kc == 0),
                        stop=False,
                    )
                nc.tensor.matmul(
                    pv4[:, t4, :], lhsT=ones_bf, rhs=b_v_sb, start=False, stop=True
                )
            nc.scalar.copy(
                v_sb[:, tw * 4:(tw + 1) * 4, :, 0:D],
                pv4.rearrange("p t (h d) -> p t h d", d=D),
            )

        def stage_b(hp, tw):
            nkc = 4 * (tw + 1)
            pvs = []
            for hh in range(2):
                pv_t = ppv.tile([D + 1, 512], f32, tag="pv", name=f"pv_{hp}_{tw}_{hh}")
                pvs.append(pv_t)
            for kc in range(nkc):
                j = kc - 4 * tw
                q0 = 128 * j if j > 0 else 0
                stp = pairs.tile([128, 2, 512], f32, tag="pair", name=f"st_{hp}_{tw}_{kc}")
                for hh in range(2):
                    h = 2 * hp + hh
                    base = 64 * (h % 2)
                    fq = h // 2
                    fk = 2 + h // 2
                    nc.tensor.matmul(
                        stp[:, hh, q0:],
                        lhsT=qkT_sb[base:base + 64, fk, kc * 128:(kc + 1) * 128],
                        rhs=qkT_sb[base:base + 64, fq, tw * 512 + q0:(tw + 1) * 512],
                        start=True,
                        stop=True,
                    )
                pt = ptp.tile([128, 2, 512], bf16, tag="pt", name=f"pt_{hp}_{tw}_{kc}")
                nc.scalar.activation(
                    out=pt[:, :, q0:], in_=stp[:, :, q0:], func=AF.Exp, scale=0.125
                )
                if j >= 0:
                    nc.vector.tensor_mul(
                        pt[:, :, 128 * j:128 * (j + 1)],
                        pt[:, :, 128 * j:128 * (j + 1)],
                        tri_sb[:, None, :].broadcast_to([128, 2, 128]),
                    )
                for hh in range(2):
                    h = 2 * hp + hh
                    nc.tensor.matmul(
                        pvs[hh][:, q0:],
                        lhsT=v_sb[:, kc, h, :],
                        rhs=pt[:, hh, q0:],
                        start=(kc == 0),
                        stop=(kc == nkc - 1),
                    )
            return pvs

        def stage_b_epi(hp, tw, pvs):
            for hh in range(2):
                h = 2 * hp + hh
                den = smalls.tile([1, 512], f32, tag="den", name=f"den_{hp}_{tw}_{hh}")
                nc.vector.tensor_copy(den, pvs[hh][D:D + 1, :])
                rec = smalls.tile([1, 512], f32, tag="rec", name=f"rec_{hp}_{tw}_{hh}")
                nc.vector.reciprocal_approx_fast(out=rec, in_=den)
                Rb = smalls.tile([64, 512], f32, tag="Rb", name=f"Rb_{hp}_{tw}_{hh}", bufs=2)
                nc.gpsimd.partition_broadcast(Rb, rec)
                nc.vector.tensor_mul(
                    z_sb[64 * (h % 2):64 * (h % 2) + 64, h // 2, tw * 512:(tw + 1) * 512],
                    pvs[hh][0:D, :],
                    Rb,
                )

        def stage_c(tt):
            psy = pairs.tile([128, 2, 512], f32, tag="pair", name=f"psy_{tt}")
            for n in range(2):
                for fc in range(2):
                    nc.tensor.matmul(
                        psy[:, n, :],
                        lhsT=z_sb[:, fc, tt * 128:(tt + 1) * 128],
                        rhs=w_p_sb[:, fc, n * 512:(n + 1) * 512],
                        start=(fc == 0),
                        stop=(fc == 1),
                    )
            yt = ytp.tile([128, 2, 512], bf16, tag="yt", name=f"yt_{tt}")
            nc.vector.tensor_copy(yt, psy)
            nc.sync.dma_start(
                out=y[tt * 128:(tt + 1) * 128, :], in_=yt.rearrange("p a b -> p (a b)")
            )

        for tw in range(NTW):
            if tw + 1 < NTW:
                prefetch_x(tw + 1)
            stage_a(tw)
            if tw > 0:
                for t4 in range(4):
                    stage_c((tw - 1) * 4 + t4)
            pvs0 = stage_b(0, tw)
            pvs1 = stage_b(1, tw)
            stage_b_epi(0, tw, pvs0)
            stage_b_epi(1, tw, pvs1)
        for t4 in range(4):
            stage_c(3 * 4 + t4)

        if debug_outputs:
            nc.sync.dma_start(out=qkT_dbg[:, :, :], in_=qkT_sb)
            nc.sync.dma_start(
                out=v_dbg[:, :, :], in_=v_sb.rearrange("p t h e -> p t (h e)")
            )
            nc.sync.dma_start(out=z_dbg[:, :, :], in_=z_sb)
    nc.finalize()
    return nc


def make_in_maps(x, W_attn, b_attn, W_proj):
    import ml_dtypes

    bf = ml_dtypes.bfloat16
    p = np.arange(128)[:, None]
    q = np.arange(128)[None, :]
    tri = (q >= p).astype(np.float32).astype(bf)
    xT = [np.ascontiguousarray(x[b].T).astype(bf) for b in range(B)]
    in_maps = []
    for c in range(8):
        b, g = c // 4, c % 4
        heads = [4 * g + i for i in range(HPC)]
        wq = np.concatenate([W_attn[:, 64 * h:64 * h + 64] for h in heads], axis=1)
        wk = np.concatenate([W_attn[:, C + 64 * h:C + 64 * h + 64] for h in heads], axis=1)
        wv = np.concatenate([W_attn[:, 2 * C + 64 * h:2 * C + 64 * h + 64] for h in heads], axis=1)
        bq = np.concatenate([b_attn[64 * h:64 * h + 64] for h in heads])
        bk = np.concatenate([b_attn[C + 64 * h:C + 64 * h + 64] for h in heads])
        bv = np.concatenate([b_attn[2 * C + 64 * h:2 * C + 64 * h + 64] for h in heads])
        in_maps.append({
            "xT": xT[b],
            "w_qk": np.ascontiguousarray(np.concatenate([wq, wk], axis=1)).astype(bf),
            "b_qk": np.ascontiguousarray(np.concatenate([bq, bk]), dtype=np.float32),
            "w_v": np.ascontiguousarray(wv).astype(bf),
            "b_v": np.ascontiguousarray(bv).astype(bf),
            "w_p": np.ascontiguousarray(W_proj[FW * g:FW * (g + 1), :]).astype(bf),
            "tri": tri,
        })
    return in_maps


def assemble(results, b_proj):
    y = np.zeros((B, T, C), np.float32)
    for c in range(8):
        y[c // 4] += np.asarray(results[c]["y"], dtype=np.float32)
    y += b_proj[None, None, :].astype(np.float32)
    return y


def kernel(**inputs):
    from concourse.bass_utils import run_bass_kernel_spmd

    x = np.asarray(inputs["x"], np.float32)
    W_attn = np.asarray(inputs["W_attn"], np.float32)
    b_attn = np.asarray(inputs["b_attn"], np.float32)
    W_proj = np.asarray(inputs["W_proj"], np.float32)
    b_proj = np.asarray(inputs["b_proj"], np.float32)

    if "nc" not in _CACHE:
        _CACHE["nc"] = _build_nc()
    nc = _CACHE["nc"]
    in_maps = make_in_maps(x, W_attn, b_attn, W_proj)
    res = run_bass_kernel_spmd(nc, in_maps, core_ids=list(range(8))).results
    return assemble(res, b_proj)
